# revision 9
# baseline (speedup 1.0000x reference)
"""Location-dependent 3D conv (AsymConv) on 8 TRN2 NeuronCores.

Math (per output voxel):
    out[b, 0, x, y, z] = sum_{i,j,l in 0..2} Xp[b, x+i, y+j, z+l] * W[x, y, z, (i*3+j)*3+l]
with Xp = edge-padded X by 1 plane on each spatial side.

Strategy:
  - Shard the X spatial axis (96 = 8 cores x 12 planes). Host slices overlapping
    halo windows (14 planes) per core -> no inter-core communication at all.
  - Per core, SBUF layout: partition dim = y (96 used of 128), free = (b, x, z).
    Compute-engine APs must start at partition 0/32/64/96, so the y-shift cannot
    be a partition offset: the host ships 3 y-pre-shifted copies of the (small)
    X shard instead. The x/z shifts are plain free-dim AP offsets.
  - Products patch*W run on the Vector engine in fp16 (2x perf mode needs
    4-byte-aligned starts, so taps with l==1 read from a z-shifted copy made
    on the otherwise-idle ScalarE; those taps are issued last to hide the copies).
  - The 27-term accumulation runs on the otherwise-idle TensorEngine as
    identity-matmuls accumulating into PSUM (fp32), freeing the Vector engine
    from the adds. A dummy matmul spin at kernel start warms the PE HAM clock
    gate during the DMA phase.
  - W (the 6 MB stream that dominates DMA) is shipped y-major in tap-issue
    order so each of its 7 chunk-DMAs moves multi-KB contiguous runs per
    partition (small per-row descriptors were capping DMA at ~160 GB/s).
  - PSUM -> SBUF (ScalarE) -> DRAM in fp32; host reassembles the full tensor.
"""

import os

import numpy as np

# ---- problem constants (hardcoded per harness rules) ----
B = 2
D = 96  # Dx = Dy = Dz
KSZ = 3
NTAP = KSZ**3  # 27
NCORES = 8
XS = D // NCORES  # 12 x-planes per core
XH = XS + 2  # with halo
ZP = D + 2  # padded z

F16 = np.float16
LAST_RESULT = None  # BassKernelResults of the most recent run (for test.py)

_GRAPH_CACHE = {}

N_WARMUP = int(os.environ.get("ASYM_WARMUP", "10"))

# taps with l != 1 are 4B-aligned in the base copies; issue them first so the
# ScalarE z-shift copies (needed by l == 1 taps) are off the critical path
TAP_ORDER = [t for t in range(NTAP) if t % 3 != 1] + [t for t in range(NTAP) if t % 3 == 1]

# W chunk sizes (in taps, along TAP_ORDER): first small so compute starts early
W_CHUNKS = [2, 3, 4, 4, 4, 5, 5]
assert sum(W_CHUNKS) == NTAP

# per-b x-chunks whose fp32 free size fits one 2KB PSUM bank
CH = [(0, 5), (5, 5), (10, 2)]


def _build_graph():
    """Build (and cache) the per-core Bass graph. Same graph for all 8 cores."""
    if "nc" in _GRAPH_CACHE:
        return _GRAPH_CACHE["nc"]

    from concourse import bacc
    import concourse.mybir as mybir
    from concourse.tile import TileContext

    f16 = mybir.dt.float16
    f32 = mybir.dt.float32

    nc = bacc.Bacc("TRN2", target_bir_lowering=False, debug=False, num_devices=NCORES)

    # y-pre-shifted X copies: xj[y', b, x, z] = Xp[y'+j, b, x, z]
    x_ds = [
        nc.dram_tensor(f"x{j}", [D, B, XH, ZP], f16, kind="ExternalInput")
        for j in range(KSZ)
    ]
    # [y, tap (TAP_ORDER-permuted), x, z] so chunk DMAs are per-partition contiguous
    w_d = nc.dram_tensor("w", [D, NTAP, XS, D], f16, kind="ExternalInput")
    id_d = nc.dram_tensor("ident", [D, D], f16, kind="ExternalInput")
    out_d = nc.dram_tensor("out", [D, B, XS, D], f32, kind="ExternalOutput")

    with TileContext(nc) as tc:
        with (
            tc.tile_pool(name="xp", bufs=1) as xpool,
            tc.tile_pool(name="wp", bufs=1) as wpool,
            tc.tile_pool(name="pp", bufs=1) as ppool,
            tc.tile_pool(name="psp", bufs=1, space="PSUM") as pspool,
        ):
            # identity first (tiny), then x0 and the first W chunk
            id_t = xpool.tile([D, D], f16, name="id_t", tag="id_t")
            nc.sync.dma_start(out=id_t[:], in_=id_d.ap())

            # PE warm-up: spin dummy matmuls during the DMA phase so the HAM
            # clock gate reaches 2.4 GHz before the real accumulation starts
            if N_WARMUP:
                dummy = ppool.tile([D, 480], f16, name="dummy", tag="warm_rhs", bufs=1)
                nc.vector.memset(dummy[:], 0.0)
                ps_w = pspool.tile([D, 480], f32, name="ps_warm", tag="ps_warm")
                for _ in range(N_WARMUP):
                    nc.tensor.matmul(ps_w[:], id_t[:], dummy[:], start=True, stop=True)

            # spread DMA issue across the three descriptor-generation paths
            # (SP-HWDGE, ACT-HWDGE, POOL-SWDGE) — a single ring feeds the 16
            # SDMA engines too slowly (~130 GB/s measured vs ~360 available)
            w_t = wpool.tile([D, NTAP, XS, D], f16, name="w_t", tag="w_t")
            x_ts = []
            s0 = 0
            w_qs = [nc.scalar, nc.gpsimd, nc.scalar, nc.gpsimd, nc.scalar, nc.gpsimd, nc.scalar]
            for j in range(KSZ):
                xt = xpool.tile([D, B, XH, ZP], f16, name=f"x_{j}", tag=f"x_{j}")
                nc.sync.dma_start(out=xt[:], in_=x_ds[j].ap())
                x_ts.append(xt)
                ntaps = W_CHUNKS[j]
                w_qs[j].dma_start(
                    out=w_t[:, s0 : s0 + ntaps], in_=w_d.ap()[:, s0 : s0 + ntaps]
                )
                s0 += ntaps
            for ci, ntaps in enumerate(W_CHUNKS[KSZ:], start=KSZ):
                w_qs[ci].dma_start(
                    out=w_t[:, s0 : s0 + ntaps], in_=w_d.ap()[:, s0 : s0 + ntaps]
                )
                s0 += ntaps

            x1_ts = []  # z-shifted by 1 (l = 1)
            for j in range(KSZ):
                x1 = xpool.tile([D, B, XH, ZP - 1], f16, name=f"xz_{j}", tag=f"xz_{j}")
                nc.scalar.copy(out=x1[:], in_=x_ts[j][:, :, :, 1:ZP])
                x1_ts.append(x1)

            for b in range(B):
                psums = [
                    pspool.tile([D, nx, D], f32, name=f"ps_{b}_{ci}", tag=f"ps_{b}_{ci}")
                    for ci, (x0, nx) in enumerate(CH)
                ]
                for tn, t in enumerate(TAP_ORDER):
                    i, j, l = t // 9, (t // 3) % 3, t % 3
                    src, le = (x_ts[j], l) if l != 1 else (x1_ts[j], 0)
                    prod = ppool.tile([D, XS, D], f16, name="prod", tag="prod", bufs=8)
                    nc.vector.tensor_mul(
                        out=prod[:],
                        in0=src[:, b, i : i + XS, le : le + D],
                        in1=w_t[:, tn, :, :],  # slot tn
                    )
                    for ci, (x0, nx) in enumerate(CH):
                        nc.tensor.matmul(
                            psums[ci][:],
                            id_t[:],
                            prod[:, x0 : x0 + nx, :],
                            start=(tn == 0),
                            stop=(tn == NTAP - 1),
                        )
                for ci, (x0, nx) in enumerate(CH):
                    outsb = ppool.tile([D, nx, D], f32, name="outsb", tag=f"outsb_{ci}")
                    nc.scalar.copy(out=outsb[:], in_=psums[ci][:])
                    nc.sync.dma_start(
                        out=out_d.ap()[:, b, x0 : x0 + nx, :],
                        in_=outsb[:],
                    )

    nc.compile()
    _GRAPH_CACHE["nc"] = nc
    return nc


def make_in_maps(X, W):
    """Host-side shard prep. X [2,1,96,96,96] f32, W [1,1,96,96,96,27] f32."""
    X = np.asarray(X)
    W = np.asarray(W)
    Xs = X.reshape(B, D, D, D)
    # edge padding on all three spatial dims
    Xp = np.pad(Xs, ((0, 0), (1, 1), (1, 1), (1, 1)), mode="edge")
    # -> [y, b, x, z]
    Xt = np.ascontiguousarray(np.transpose(Xp, (2, 0, 1, 3))).astype(F16)
    W00 = W.reshape(D, D, D, NTAP)
    ident = np.eye(D, dtype=F16)

    in_maps = []
    for m in range(NCORES):
        xs_full = Xt[:, :, m * XS : m * XS + XH, :]  # [98, 2, 14, 98]
        im = {"ident": ident}
        for j in range(KSZ):
            im[f"x{j}"] = np.ascontiguousarray(xs_full[j : j + D])
        wm = W00[m * XS : (m + 1) * XS]  # [12, 96, 96, 27]
        # [y, tap, x, z] with taps permuted into issue order
        wm = np.transpose(wm, (1, 3, 0, 2))[:, TAP_ORDER]
        im["w"] = np.ascontiguousarray(wm).astype(F16)
        in_maps.append(im)
    return in_maps


def kernel(X, W):
    global LAST_RESULT
    from concourse.bass_utils import run_bass_kernel_spmd

    nc = _build_graph()
    in_maps = make_in_maps(X, W)
    trace = bool(int(os.environ.get("ASYM_TRACE", "0")))
    res = run_bass_kernel_spmd(
        nc, in_maps, core_ids=list(range(NCORES)), trace=trace
    )
    LAST_RESULT = res

    out = np.empty((B, 1, D, D, D), dtype=np.float32)
    for m in range(NCORES):
        r = res.results[m]["out"]  # [y, b, x, z] f32
        out[:, 0, m * XS : (m + 1) * XS, :, :] = np.transpose(r, (1, 2, 0, 3))
    return out


# revision 11
# speedup vs baseline: 1.1222x; 1.1222x over previous
"""Location-dependent 3D conv (AsymConv) on 8 TRN2 NeuronCores.

Math (per output voxel):
    out[b, 0, x, y, z] = sum_{i,j,l in 0..2} Xp[b, x+i, y+j, z+l] * W[x, y, z, (i*3+j)*3+l]
with Xp = edge-padded X by 1 plane on each spatial side.

Strategy:
  - Shard the X spatial axis (96 = 8 cores x 12 planes). Host slices overlapping
    halo windows (14 planes) per core -> no inter-core communication at all.
  - Per core, SBUF layout: partition dim = y (96 used of 128), free = (b, x, z).
    Compute-engine APs must start at partition 0/32/64/96, so the y-shift cannot
    be a partition offset: the host ships 3 y-pre-shifted copies of the (small)
    X shard instead. The x/z shifts are plain free-dim AP offsets.
  - Products patch*W run on the Vector engine in fp16 (2x perf mode needs
    4-byte-aligned starts, so taps with l==1 read from a z-shifted copy made
    on the otherwise-idle ScalarE; those taps are issued last to hide the copies).
  - The 27-term accumulation runs on the otherwise-idle TensorEngine as
    identity-matmuls accumulating into PSUM (fp32), freeing the Vector engine
    from the adds. A dummy matmul spin at kernel start warms the PE HAM clock
    gate during the DMA phase.
  - W (the 6 MB stream that dominates DMA) is shipped y-major in tap-issue
    order so each of its 7 chunk-DMAs moves multi-KB contiguous runs per
    partition (small per-row descriptors were capping DMA at ~160 GB/s).
  - PSUM -> SBUF (ScalarE) -> DRAM in fp32; host reassembles the full tensor.
"""

import os

import numpy as np

# ---- problem constants (hardcoded per harness rules) ----
B = 2
D = 96  # Dx = Dy = Dz
KSZ = 3
NTAP = KSZ**3  # 27
NCORES = 8
XS = D // NCORES  # 12 x-planes per core
XH = XS + 2  # with halo
ZP = D + 2  # padded z

F16 = np.float16
LAST_RESULT = None  # BassKernelResults of the most recent run (for test.py)

_GRAPH_CACHE = {}

N_WARMUP = int(os.environ.get("ASYM_WARMUP", "10"))

# taps with l != 1 are 4B-aligned in the base copies; issue them first so the
# ScalarE z-shift copies (needed by l == 1 taps) are off the critical path
TAP_ORDER = [t for t in range(NTAP) if t % 3 != 1] + [t for t in range(NTAP) if t % 3 == 1]

# W chunk sizes (in taps, along TAP_ORDER): first small so compute starts early
W_CHUNKS = [2, 3, 4, 4, 4, 5, 5]
assert sum(W_CHUNKS) == NTAP

# per-b x-chunks whose fp32 free size fits one 2KB PSUM bank
CH = [(0, 5), (5, 5), (10, 2)]


def _build_graph():
    """Build (and cache) the per-core Bass graph. Same graph for all 8 cores."""
    if "nc" in _GRAPH_CACHE:
        return _GRAPH_CACHE["nc"]

    from concourse import bacc
    import concourse.mybir as mybir
    from concourse.tile import TileContext

    f16 = mybir.dt.float16
    f32 = mybir.dt.float32

    nc = bacc.Bacc("TRN2", target_bir_lowering=False, debug=False, num_devices=NCORES)

    # y-pre-shifted X copies: xj[y', b, x, z] = Xp[y'+j, b, x, z]
    x_ds = [
        nc.dram_tensor(f"x{j}", [D, B, XH, ZP], f16, kind="ExternalInput")
        for j in range(KSZ)
    ]
    # [y, tap (TAP_ORDER-permuted), x, z] so chunk DMAs are per-partition contiguous
    w_d = nc.dram_tensor("w", [D, NTAP, XS, D], f16, kind="ExternalInput")
    id_d = nc.dram_tensor("ident", [D, D], f16, kind="ExternalInput")
    out_d = nc.dram_tensor("out", [D, B, XS, D], f32, kind="ExternalOutput")

    with TileContext(nc) as tc:
        with (
            tc.tile_pool(name="xp", bufs=1) as xpool,
            tc.tile_pool(name="wp", bufs=1) as wpool,
            tc.tile_pool(name="pp", bufs=1) as ppool,
            tc.tile_pool(name="psp", bufs=1, space="PSUM") as pspool,
        ):
            # identity first (tiny), then x0 and the first W chunk
            id_t = xpool.tile([D, D], f16, name="id_t", tag="id_t")
            nc.sync.dma_start(out=id_t[:], in_=id_d.ap())

            # PE warm-up: spin dummy matmuls during the DMA phase so the HAM
            # clock gate reaches 2.4 GHz before the real accumulation starts
            if N_WARMUP:
                dummy = ppool.tile([D, 480], f16, name="dummy", tag="warm_rhs", bufs=1)
                nc.vector.memset(dummy[:], 0.0)
                ps_w = pspool.tile([D, 480], f32, name="ps_warm", tag="ps_warm")
                for _ in range(N_WARMUP):
                    nc.tensor.matmul(ps_w[:], id_t[:], dummy[:], start=True, stop=True)

            # spread DMA issue across the three descriptor-generation paths
            # (SP-HWDGE, ACT-HWDGE, POOL-SWDGE) — a single ring feeds the 16
            # SDMA engines too slowly (~130 GB/s measured vs ~360 available)
            # The first small W chunk goes on the gpsimd (SWDGE) ring — it is
            # done before DVE compute starts, so no descriptor-ring/2-port
            # contention. The rest alternates between the two HWDGE rings
            # (SP = nc.sync, ACT = nc.scalar); each ring feeds only ~100 GB/s.
            w_t = wpool.tile([D, NTAP, XS, D], f16, name="w_t", tag="w_t")
            x_ts = []
            s0 = 0
            w_qs = [nc.gpsimd, nc.scalar, nc.scalar, nc.sync, nc.scalar, nc.sync, nc.scalar]
            for j in range(KSZ):
                xt = xpool.tile([D, B, XH, ZP], f16, name=f"x_{j}", tag=f"x_{j}")
                nc.sync.dma_start(out=xt[:], in_=x_ds[j].ap())
                x_ts.append(xt)
                ntaps = W_CHUNKS[j]
                w_qs[j].dma_start(
                    out=w_t[:, s0 : s0 + ntaps], in_=w_d.ap()[:, s0 : s0 + ntaps]
                )
                s0 += ntaps
            for ci, ntaps in enumerate(W_CHUNKS[KSZ:], start=KSZ):
                w_qs[ci].dma_start(
                    out=w_t[:, s0 : s0 + ntaps], in_=w_d.ap()[:, s0 : s0 + ntaps]
                )
                s0 += ntaps

            x1_ts = []  # z-shifted by 1 (l = 1)
            for j in range(KSZ):
                x1 = xpool.tile([D, B, XH, ZP - 1], f16, name=f"xz_{j}", tag=f"xz_{j}")
                nc.scalar.copy(out=x1[:], in_=x_ts[j][:, :, :, 1:ZP])
                x1_ts.append(x1)

            psums = {
                (b, ci): pspool.tile(
                    [D, nx, D], f32, name=f"ps_{b}_{ci}", tag=f"ps_{b}_{ci}"
                )
                for b in range(B)
                for ci, (x0, nx) in enumerate(CH)
            }
            # tap-major: each W slot is consumed for both b right after it
            # lands, so DVE never races ahead of the W stream
            for tn, t in enumerate(TAP_ORDER):
                i, j, l = t // 9, (t // 3) % 3, t % 3
                src, le = (x_ts[j], l) if l != 1 else (x1_ts[j], 0)
                for b in range(B):
                    prod = ppool.tile([D, XS, D], f16, name="prod", tag="prod", bufs=8)
                    nc.vector.tensor_mul(
                        out=prod[:],
                        in0=src[:, b, i : i + XS, le : le + D],
                        in1=w_t[:, tn, :, :],  # slot tn
                    )
                    for ci, (x0, nx) in enumerate(CH):
                        nc.tensor.matmul(
                            psums[(b, ci)][:],
                            id_t[:],
                            prod[:, x0 : x0 + nx, :],
                            start=(tn == 0),
                            stop=(tn == NTAP - 1),
                        )
            for b in range(B):
                for ci, (x0, nx) in enumerate(CH):
                    outsb = ppool.tile(
                        [D, nx, D], f32, name="outsb", tag=f"outsb_{b}_{ci}"
                    )
                    nc.scalar.copy(out=outsb[:], in_=psums[(b, ci)][:])
                    nc.sync.dma_start(
                        out=out_d.ap()[:, b, x0 : x0 + nx, :],
                        in_=outsb[:],
                    )

    nc.compile()
    _GRAPH_CACHE["nc"] = nc
    return nc


def make_in_maps(X, W):
    """Host-side shard prep. X [2,1,96,96,96] f32, W [1,1,96,96,96,27] f32."""
    X = np.asarray(X)
    W = np.asarray(W)
    Xs = X.reshape(B, D, D, D)
    # edge padding on all three spatial dims
    Xp = np.pad(Xs, ((0, 0), (1, 1), (1, 1), (1, 1)), mode="edge")
    # -> [y, b, x, z]
    Xt = np.ascontiguousarray(np.transpose(Xp, (2, 0, 1, 3))).astype(F16)
    W00 = W.reshape(D, D, D, NTAP)
    ident = np.eye(D, dtype=F16)

    in_maps = []
    for m in range(NCORES):
        xs_full = Xt[:, :, m * XS : m * XS + XH, :]  # [98, 2, 14, 98]
        im = {"ident": ident}
        for j in range(KSZ):
            im[f"x{j}"] = np.ascontiguousarray(xs_full[j : j + D])
        wm = W00[m * XS : (m + 1) * XS]  # [12, 96, 96, 27]
        # [y, tap, x, z] with taps permuted into issue order
        wm = np.transpose(wm, (1, 3, 0, 2))[:, TAP_ORDER]
        im["w"] = np.ascontiguousarray(wm).astype(F16)
        in_maps.append(im)
    return in_maps


def kernel(X, W):
    global LAST_RESULT
    from concourse.bass_utils import run_bass_kernel_spmd

    nc = _build_graph()
    in_maps = make_in_maps(X, W)
    trace = bool(int(os.environ.get("ASYM_TRACE", "0")))
    res = run_bass_kernel_spmd(
        nc, in_maps, core_ids=list(range(NCORES)), trace=trace
    )
    LAST_RESULT = res

    out = np.empty((B, 1, D, D, D), dtype=np.float32)
    for m in range(NCORES):
        r = res.results[m]["out"]  # [y, b, x, z] f32
        out[:, 0, m * XS : (m + 1) * XS, :, :] = np.transpose(r, (1, 2, 0, 3))
    return out


# revision 14
# speedup vs baseline: 1.2225x; 1.0894x over previous
"""Location-dependent 3D conv (AsymConv) on 8 TRN2 NeuronCores.

Math (per output voxel):
    out[b, 0, x, y, z] = sum_{i,j,l in 0..2} Xp[b, x+i, y+j, z+l] * W[x, y, z, (i*3+j)*3+l]
with Xp = edge-padded X by 1 plane on each spatial side.

Strategy:
  - Shard the X spatial axis (96 = 8 cores x 12 planes). Host slices overlapping
    halo windows (14 planes) per core -> no inter-core communication at all.
  - Per core, SBUF layout: partition dim = y (96 used of 128), free = (b, x, z).
    Compute-engine APs must start at partition 0/32/64/96, so the y-shift cannot
    be a partition offset: one padded X shard (98 partitions) is shipped and the
    j = 1, 2 y-shifted copies are derived with partition-shifted SBUF->SBUF DMAs
    (DMA has no partition-start restriction). The x/z shifts are free-dim offsets.
  - Products patch*W run on the Vector engine in fp16 (2x perf mode needs
    4-byte-aligned starts, so taps with l==1 read from a z-shifted copy made
    on the otherwise-idle ScalarE; those taps are issued last to hide the copies).
  - The 27-term accumulation runs on the otherwise-idle TensorEngine as
    identity-matmuls accumulating into PSUM (fp32). A dummy matmul spin at
    kernel start warms the PE HAM clock gate during the DMA phase.
  - W (the 6 MB stream that dominates DMA) is shipped y-major in tap-issue
    order and split over BOTH HWDGE rings (SP = nc.sync, ACT = nc.scalar);
    a single ring feeds the 16 SDMA engines at only ~130 GB/s.
  - b-outer loop: batch 0's accumulation finishes halfway through and its
    PSUM flush overlaps batch 1's compute.
  - PSUM -> SBUF (ScalarE) -> DRAM in fp32; host reassembles the full tensor.
"""

import os

import numpy as np

# ---- problem constants (hardcoded per harness rules) ----
B = 2
D = 96  # Dx = Dy = Dz
KSZ = 3
NTAP = KSZ**3  # 27
NCORES = 8
XS = D // NCORES  # 12 x-planes per core
XH = XS + 2  # with halo
YP = D + 2  # padded y (partition dim of the shipped X shard)
ZP = D + 2  # padded z

F16 = np.float16
LAST_RESULT = None  # BassKernelResults of the most recent run (for test.py)

_GRAPH_CACHE = {}

N_WARMUP = int(os.environ.get("ASYM_WARMUP", "10"))

# taps with l != 1 are 4B-aligned in the base copies; issue them first so the
# ScalarE z-shift copies (needed by l == 1 taps) are off the critical path
TAP_ORDER = [t for t in range(NTAP) if t % 3 != 1] + [t for t in range(NTAP) if t % 3 == 1]

# W chunk sizes (in taps, along TAP_ORDER): first small so compute starts early
W_CHUNKS = [2, 3, 4, 4, 4, 5, 5]
assert sum(W_CHUNKS) == NTAP

# per-b x-chunks whose fp32 free size fits one 2KB PSUM bank
CH = [(0, 5), (5, 5), (10, 2)]


def _build_graph():
    """Build (and cache) the per-core Bass graph. Same graph for all 8 cores."""
    if "nc" in _GRAPH_CACHE:
        return _GRAPH_CACHE["nc"]

    from concourse import bacc
    import concourse.mybir as mybir
    from concourse.tile import TileContext

    f16 = mybir.dt.float16
    f32 = mybir.dt.float32

    nc = bacc.Bacc("TRN2", target_bir_lowering=False, debug=False, num_devices=NCORES)

    x_d = nc.dram_tensor("x", [YP, B, XH, ZP], f16, kind="ExternalInput")
    # [y, tap (TAP_ORDER-permuted), x, z] so chunk DMAs are per-partition contiguous
    w_d = nc.dram_tensor("w", [D, NTAP, XS, D], f16, kind="ExternalInput")
    id_d = nc.dram_tensor("ident", [D, D], f16, kind="ExternalInput")
    out_d = nc.dram_tensor("out", [D, B, XS, D], f32, kind="ExternalOutput")

    with TileContext(nc) as tc:
        with (
            tc.tile_pool(name="xp", bufs=1) as xpool,
            tc.tile_pool(name="wp", bufs=1) as wpool,
            tc.tile_pool(name="pp", bufs=1) as ppool,
            tc.tile_pool(name="psp", bufs=1, space="PSUM") as pspool,
        ):
            id_t = xpool.tile([D, D], f16, name="id_t", tag="id_t")
            nc.sync.dma_start(out=id_t[:], in_=id_d.ap())

            # PE warm-up: spin dummy matmuls during the DMA phase so the HAM
            # clock gate reaches 2.4 GHz before the real accumulation starts
            if N_WARMUP:
                dummy = ppool.tile([D, 480], f16, name="dummy", tag="warm_rhs", bufs=1)
                nc.vector.memset(dummy[:], 0.0)
                ps_w = pspool.tile([D, 480], f32, name="ps_warm", tag="ps_warm")
                for _ in range(N_WARMUP):
                    nc.tensor.matmul(ps_w[:], id_t[:], dummy[:], start=True, stop=True)

            # one padded X shard; y-shifted copies derived on-device
            x_full = xpool.tile([YP, B, XH, ZP], f16, name="x_full", tag="x_full")
            nc.sync.dma_start(out=x_full[:], in_=x_d.ap())
            x_sh = [x_full]
            for j in (1, 2):
                xj = xpool.tile([D, B, XH, ZP], f16, name=f"xs_{j}", tag=f"xs_{j}")
                nc.sync.dma_start(out=xj[:], in_=x_full[j : j + D])
                x_sh.append(xj)

            # W over both HWDGE rings, first chunk on the (otherwise busy
            # with X) sync ring's sibling so tap 0 starts ASAP
            w_t = wpool.tile([D, NTAP, XS, D], f16, name="w_t", tag="w_t")
            w_qs = [nc.scalar, nc.scalar, nc.sync, nc.scalar, nc.sync, nc.scalar, nc.sync]
            s0 = 0
            for ci, ntaps in enumerate(W_CHUNKS):
                w_qs[ci].dma_start(
                    out=w_t[:, s0 : s0 + ntaps], in_=w_d.ap()[:, s0 : s0 + ntaps]
                )
                s0 += ntaps

            x1_ts = []  # z-shifted by 1 (l = 1), built on ScalarE
            for j in range(KSZ):
                x1 = xpool.tile([D, B, XH, ZP - 1], f16, name=f"xz_{j}", tag=f"xz_{j}")
                nc.scalar.copy(out=x1[:], in_=x_sh[j][0:D, :, :, 1:ZP])
                x1_ts.append(x1)

            for b in range(B):
                psums = [
                    pspool.tile([D, nx, D], f32, name=f"ps_{b}_{ci}", tag=f"ps_{b}_{ci}")
                    for ci, (x0, nx) in enumerate(CH)
                ]
                for tn, t in enumerate(TAP_ORDER):
                    i, j, l = t // 9, (t // 3) % 3, t % 3
                    src, le = (x_sh[j], l) if l != 1 else (x1_ts[j], 0)
                    prod = ppool.tile([D, XS, D], f16, name="prod", tag="prod", bufs=8)
                    nc.vector.tensor_mul(
                        out=prod[:],
                        in0=src[0:D, b, i : i + XS, le : le + D],
                        in1=w_t[:, tn, :, :],  # slot tn
                    )
                    for ci, (x0, nx) in enumerate(CH):
                        nc.tensor.matmul(
                            psums[ci][:],
                            id_t[:],
                            prod[:, x0 : x0 + nx, :],
                            start=(tn == 0),
                            stop=(tn == NTAP - 1),
                        )
                for ci, (x0, nx) in enumerate(CH):
                    outsb = ppool.tile(
                        [D, nx, D], f32, name="outsb", tag=f"outsb_{b}_{ci}"
                    )
                    nc.scalar.copy(out=outsb[:], in_=psums[ci][:])
                    nc.sync.dma_start(
                        out=out_d.ap()[:, b, x0 : x0 + nx, :],
                        in_=outsb[:],
                    )

    nc.compile()
    _GRAPH_CACHE["nc"] = nc
    return nc


def make_in_maps(X, W):
    """Host-side shard prep. X [2,1,96,96,96] f32, W [1,1,96,96,96,27] f32."""
    X = np.asarray(X)
    W = np.asarray(W)
    Xs = X.reshape(B, D, D, D)
    # edge padding on all three spatial dims
    Xp = np.pad(Xs, ((0, 0), (1, 1), (1, 1), (1, 1)), mode="edge")
    # -> [y, b, x, z]
    Xt = np.ascontiguousarray(np.transpose(Xp, (2, 0, 1, 3))).astype(F16)
    W00 = W.reshape(D, D, D, NTAP)
    ident = np.eye(D, dtype=F16)

    in_maps = []
    for m in range(NCORES):
        im = {"ident": ident}
        im["x"] = np.ascontiguousarray(Xt[:, :, m * XS : m * XS + XH, :])
        wm = W00[m * XS : (m + 1) * XS]  # [12, 96, 96, 27]
        # [y, tap, x, z] with taps permuted into issue order
        wm = np.transpose(wm, (1, 3, 0, 2))[:, TAP_ORDER]
        im["w"] = np.ascontiguousarray(wm).astype(F16)
        in_maps.append(im)
    return in_maps


def kernel(X, W):
    global LAST_RESULT
    from concourse.bass_utils import run_bass_kernel_spmd

    nc = _build_graph()
    in_maps = make_in_maps(X, W)
    trace = bool(int(os.environ.get("ASYM_TRACE", "0")))
    res = run_bass_kernel_spmd(
        nc, in_maps, core_ids=list(range(NCORES)), trace=trace
    )
    LAST_RESULT = res

    out = np.empty((B, 1, D, D, D), dtype=np.float32)
    for m in range(NCORES):
        r = res.results[m]["out"]  # [y, b, x, z] f32
        out[:, 0, m * XS : (m + 1) * XS, :, :] = np.transpose(r, (1, 2, 0, 3))
    return out
